# revision 9
# baseline (speedup 1.0000x reference)
# Trainium2 Bass kernel for the ContextBlock problem.
#
# Reference computation (per sample b):
#   xc    = concat(x0..x3)            [C=1024, HW=4096]
#   attn  = softmax(wm @ xc)          [HW]
#   ctx   = xc @ attn                 [C]
#   mul   = residual-gated MLP stack (sigmoid branch)   [C]
#   add   = residual-gated MLP stack (linear branch)    [C]
#   out   = sum_l (x_l * mul_l + add_l)                 [CL=256, HW]
#
# Distribution: data-parallel over batch, one sample per NeuronCore (B=8).
#
# Per-core dataflow (v2.2):
#   x arrives in 4 column-blocks of 1024; per block: PE computes logits,
#   scalar exps them (unnormalized: the softmax scale cancels through the
#   gates' LayerNorm), PE broadcasts e across partitions, DVE+Scalar
#   accumulate u[c] = sum_n x[c,n] e[n].
#   Gates in fp8 (weights pre-scaled; LN washes the scale): both branches
#   fused per repeat, weight-stationary matvecs on PE, LN stats via gpsimd
#   partition_all_reduce, sigmoid as a degree-7 odd polynomial on DVE
#   (avoids activation-table swaps; |z| < 2 in practice).
#   Pass3: out chunks split PE (diag-matmul) / Scalar+DVE (scalar chains).

import numpy as np
import ml_dtypes
from contextlib import ExitStack

import concourse.bass as bass
import concourse.bacc as bacc
import concourse.mybir as mybir
import concourse.tile as tile

BF = mybir.dt.bfloat16
F8 = mybir.dt.float8e4
F32 = mybir.dt.float32
AF = mybir.ActivationFunctionType
ALU = mybir.AluOpType
AX = mybir.AxisListType

B, L, CL, H, W = 8, 4, 256, 64, 64
C = L * CL          # 1024
HW = H * W          # 4096
P = C // 4          # 256
R = 2
EPS = 1e-5
NJ = C // 128       # 8   c-slabs
NBLK = 4            # x column blocks of 1024
BLKW = HW // NBLK   # 1024
NCORES = 8

# fp8 scaling: h is computed as SC_H * (W1 @ v); LN is scale-invariant.
SC_V0 = 0.25        # v0 (context) pre-scale into fp8 range
SC_W1R0 = 128.0     # W1 r0 host scale (SC_V0 * SC_W1R0 = SC_H = 32)
SC_V1 = 16.0        # v1 (maps) pre-scale
SC_W1R1 = 2.0       # W1 r1 host scale
SC_W2 = 32.0        # W2 host scale; z unscaled by 1/SC_W2 on-chip

# sigmoid(z) - 0.5 ~= ((d z^2 + c) z^2 + b) z^2 + a) * z  on |z|<3.5
SGA, SGB, SGC, SGD = 2.48367550e-01, -1.85723651e-02, 1.15603304e-03, -3.27830204e-05

_CACHE = {}


def _build_nc():
    import concourse.bass_isa as bass_isa

    nc = bacc.Bacc()

    x_d = nc.dram_tensor("x", [C, HW], BF, kind="ExternalInput")
    wmc_d = nc.dram_tensor("wmc", [128, NJ], BF, kind="ExternalInput")
    rhsi_d = nc.dram_tensor("rhsi", [128, 128], BF, kind="ExternalInput")
    onesr_d = nc.dram_tensor("onesr", [1, 128], BF, kind="ExternalInput")
    sm_d = nc.dram_tensor("smalls", [128, 128], F32, kind="ExternalInput")
    wg1_d = nc.dram_tensor("wg1", [R, NJ, 128, 2048], F8, kind="ExternalInput")
    wg2_d = nc.dram_tensor("wg2", [R, 128, 4096], F8, kind="ExternalInput")
    out_d = nc.dram_tensor("out", [CL, HW], F32, kind="ExternalOutput")

    with tile.TileContext(nc) as tc, ExitStack() as ctx:
        resid = ctx.enter_context(tc.tile_pool(name="resid", bufs=1))
        spool = ctx.enter_context(tc.tile_pool(name="spool", bufs=1))
        stpool = ctx.enter_context(tc.tile_pool(name="stage", bufs=6))
        apool = ctx.enter_context(tc.tile_pool(name="accp", bufs=2))

        # ---- resident tiles + loads (all weights resident) ----
        wmc = resid.tile([128, NJ], BF, tag="wmc")
        rhsi = resid.tile([128, 128], BF, tag="rhsi")
        onesr = resid.tile([1, 128], BF, tag="onesr")
        sm = resid.tile([128, 128], F32, tag="sm")
        x_sb = resid.tile([128, NJ, HW], BF, tag="x")
        wg1 = resid.tile([128, R, NJ, 2048], F8, tag="wg1")
        wg2 = resid.tile([128, R, 4096], F8, tag="wg2")

        nc.sync.dma_start(wmc[:], wmc_d[:])
        nc.sync.dma_start(rhsi[:], rhsi_d[:])
        nc.sync.dma_start(onesr[:], onesr_d[:])
        nc.sync.dma_start(sm[:], sm_d[:])
        for blk in range(NBLK):
            cols = slice(BLKW * blk, BLKW * (blk + 1))
            for j in range(NJ):
                nc.sync.dma_start(x_sb[:, j, cols], x_d[128 * j:128 * (j + 1), cols])
        for r in range(R):
            for j in range(NJ):
                nc.sync.dma_start(wg1[:, r, j, :], wg1_d[r, j])
            nc.sync.dma_start(wg2[:, r, :], wg2_d[r])

        # single Exp table load up front (Identity/Copy live in every table)
        warm = spool.tile([1, 2], F32, tag="warm")
        nc.vector.memset(warm[:], 0.25)
        nc.scalar.activation(warm[0:1, 1:2], warm[0:1, 0:1], AF.Exp)

        # ---- phase A: logits -> exp -> u accumulation, per column block ---
        e_row = spool.tile([1, HW], BF, tag="e_row")
        e_bc = spool.tile([128, NBLK, BLKW], BF, tag="e_bc")
        scr_v = spool.tile([128, 2, BLKW], BF, tag="scr_v")
        prod = spool.tile([128, 2, 5, BLKW], BF, tag="prod")
        scr_s = spool.tile([128, 2, BLKW], BF, tag="scr_s")
        u_parts = spool.tile([128, NJ * NBLK], F32, tag="u_parts")

        with tc.tile_pool(name="psA", bufs=2, space=bass.MemorySpace.PSUM) as psA:
            for blk in range(NBLK):
                cols = slice(BLKW * blk, BLKW * (blk + 1))
                lg = psA.tile([1, BLKW], F32, tag="lg")
                for j in range(NJ):
                    for h in range(2):
                        nc.tensor.matmul(
                            lg[0:1, 512 * h:512 * (h + 1)],
                            wmc[:, j:j + 1],
                            x_sb[:, j, BLKW * blk + 512 * h:BLKW * blk + 512 * (h + 1)],
                            start=(j == 0), stop=(j == NJ - 1),
                        )
                nc.scalar.activation(e_row[0:1, cols], lg[:], AF.Exp)
                for h in range(2):
                    bc_ps = psA.tile([128, 512], F32, tag="bc")
                    nc.tensor.matmul(
                        bc_ps[:], onesr[:],
                        e_row[0:1, BLKW * blk + 512 * h:BLKW * blk + 512 * (h + 1)],
                    )
                    nc.scalar.copy(
                        e_bc[:, blk, 512 * h:512 * (h + 1)], bc_ps[:]
                    )
                # u accumulation: slabs 0-2 fused STT on DVE;
                # slabs 3-7: 2x tensor_tensor on DVE + accum on Scalar
                for j in range(3):
                    nc.vector.scalar_tensor_tensor(
                        out=scr_v[:, blk % 2, :],
                        in0=x_sb[:, j, cols],
                        scalar=1.0,
                        in1=e_bc[:, blk, :],
                        op0=ALU.bypass,
                        op1=ALU.mult,
                        accum_out=u_parts[:, j * NBLK + blk:j * NBLK + blk + 1],
                    )
                for j in range(3, NJ):
                    nc.vector.tensor_mul(
                        prod[:, blk % 2, j - 3, :], x_sb[:, j, cols], e_bc[:, blk, :]
                    )
                    nc.scalar.activation(
                        scr_s[:, blk % 2, :], prod[:, blk % 2, j - 3, :], AF.Identity,
                        accum_out=u_parts[:, j * NBLK + blk:j * NBLK + blk + 1],
                    )

        # prefetch the sqrt table while W1-r0 runs (Identity stays resident)
        nc.scalar.activation(warm[0:1, 1:2], warm[0:1, 0:1], AF.Sqrt)

        v0 = spool.tile([128, NJ], F32, tag="v0")
        nc.vector.reduce_sum(
            out=v0[:],
            in_=u_parts[:].rearrange("p (j b) -> p j b", b=NBLK),
            axis=AX.X,
        )
        v0_f8 = spool.tile([128, NJ], F8, tag="v0f8")
        nc.vector.tensor_scalar_mul(v0_f8[:], v0[:], SC_V0)

        # ---- gates: both branches fused per repeat, fp8 matvecs ----
        # t-col layout: t = br*8 + 2*lv + half
        ps = ctx.enter_context(
            tc.tile_pool(name="psG", bufs=2, space=bass.MemorySpace.PSUM)
        )

        def sigmoid_m05(z, tagp):
            # returns sigma(z) - 0.5 via odd poly (DVE only)
            t = spool.tile([128, 4, NJ], F32, tag=tagp)
            nc.vector.tensor_mul(t[:, 0, :], z, z)                      # z2
            nc.vector.tensor_scalar(
                out=t[:, 1, :], in0=t[:, 0, :],
                scalar1=SGD, scalar2=SGC, op0=ALU.mult, op1=ALU.add,
            )
            nc.vector.tensor_mul(t[:, 2, :], t[:, 1, :], t[:, 0, :])
            nc.vector.tensor_scalar_add(t[:, 2, :], t[:, 2, :], SGB)
            nc.vector.tensor_mul(t[:, 3, :], t[:, 2, :], t[:, 0, :])
            out = spool.tile([128, NJ], F32, tag=tagp + "o")
            nc.vector.scalar_tensor_tensor(
                out=out[:], in0=t[:, 3, :], scalar=SGA, in1=z,
                op0=ALU.add, op1=ALU.mult,
            )
            return out

        def gate_repeat(r, moving):
            b1c = sm[:, 0 + 16 * r:16 + 16 * r]
            gc = sm[:, 32 + 16 * r:48 + 16 * r]
            bec = sm[:, 64 + 16 * r:80 + 16 * r]
            b2c = sm[:, 96 + 16 * r:112 + 16 * r]

            ps_h = ps.tile([128, 16], F32, tag="ps_h")
            for j in range(NJ):
                for t in range(16):
                    nc.tensor.matmul(
                        ps_h[:, t:t + 1],
                        wg1[:, r, j, 128 * t:128 * (t + 1)],
                        moving(j, t // 8),
                        start=(j == 0 and t == 0),
                        stop=(j == NJ - 1 and t == 15),
                    )

            # LN over (partitions x 2 t-cols) per group g = br*4+lv
            stats = spool.tile([128, 32], F32, tag="stats", bufs=2)
            nc.vector.tensor_add(stats[:, 0:16], ps_h[:], b1c)
            nc.vector.tensor_mul(stats[:, 16:32], stats[:, 0:16], stats[:, 0:16])

            allred = spool.tile([128, 32], F32, tag="allred", bufs=2)
            nc.gpsimd.partition_all_reduce(
                allred[:], stats[:], channels=128, reduce_op=bass_isa.ReduceOp.add
            )

            gm = spool.tile([128, 16], F32, tag="gm", bufs=2)
            nc.vector.reduce_sum(
                out=gm[:],
                in_=allred[:].rearrange("p (g h) -> p g h", h=2),
                axis=AX.X,
            )
            nc.vector.tensor_scalar_mul(gm[:], gm[:], 1.0 / P)
            nbc = spool.tile([128, 16], F32, tag="nbc", bufs=2)
            nc.vector.tensor_mul(nbc[:, 0:8], gm[:, 0:8], gm[:, 0:8])
            nc.vector.tensor_sub(nbc[:, 8:16], gm[:, 8:16], nbc[:, 0:8])
            nc.vector.tensor_scalar_add(nbc[:, 8:16], nbc[:, 8:16], EPS)
            nc.scalar.activation(nbc[:, 0:8], nbc[:, 8:16], AF.Sqrt)
            nc.vector.reciprocal(nbc[:, 8:16], nbc[:, 0:8])
            mu_t = spool.tile([128, 32], F32, tag="mu_t", bufs=2)
            mtv = mu_t[:, 0:16].rearrange("p (g h) -> p h g", h=2)
            rtv = mu_t[:, 16:32].rearrange("p (g h) -> p h g", h=2)
            for hh in range(2):
                nc.vector.tensor_copy(mtv[:, hh, :], gm[:, 0:8])
                nc.vector.tensor_copy(rtv[:, hh, :], nbc[:, 8:16])

            hn = spool.tile([128, 16], F32, tag="hn", bufs=2)
            nc.vector.tensor_sub(hn[:], stats[:, 0:16], mu_t[:, 0:16])
            nc.vector.tensor_mul(hn[:], hn[:], mu_t[:, 16:32])
            nc.vector.tensor_mul(hn[:], hn[:], gc)
            nc.vector.tensor_add(hn[:], hn[:], bec)
            hn_f8 = spool.tile([128, 16], F8, tag="hnf8", bufs=2)
            nc.vector.tensor_scalar_max(hn_f8[:], hn[:], 0.0)

            ps_z = ps.tile([128, 16], F32, tag="ps_z")
            nblks = 0
            for br in range(2):
                for lv in range(4):
                    for clc in range(2):
                        tcol = br * 8 + 2 * lv + clc
                        for kc in range(2):
                            off = (((br * 4 + lv) * 2 + clc) * 2 + kc) * 128
                            nc.tensor.matmul(
                                ps_z[:, tcol:tcol + 1],
                                wg2[:, r, off:off + 128],
                                hn_f8[:, br * 8 + 2 * lv + kc:br * 8 + 2 * lv + kc + 1],
                                start=(nblks == 0),
                                stop=(nblks == 31),
                            )
                            nblks += 1
            zb = spool.tile([128, 16], F32, tag=f"zb{r}")
            nc.vector.scalar_tensor_tensor(
                out=zb[:], in0=ps_z[:], scalar=1.0 / SC_W2, in1=b2c,
                op0=ALU.mult, op1=ALU.add,
            )
            return zb

        zb0 = gate_repeat(0, lambda j, br: v0_f8[:, j:j + 1])

        sg0 = sigmoid_m05(zb0[:, 0:8], "sg0")          # sigma(z_mul0) - 0.5
        vmul0 = spool.tile([128, NJ], F32, tag="vmul0")
        nc.vector.tensor_scalar_add(vmul0[:], sg0[:], 0.5)
        v1_f8 = spool.tile([128, 16], F8, tag="v1f8")
        nc.vector.tensor_scalar_mul(v1_f8[:, 0:8], vmul0[:], SC_V1)
        nc.vector.tensor_scalar_mul(v1_f8[:, 8:16], zb0[:, 8:16], SC_V1)

        zb1 = gate_repeat(1, lambda j, br: v1_f8[:, br * 8 + j:br * 8 + j + 1])

        sg1 = sigmoid_m05(zb1[:, 0:8], "sg1")
        mm_f = spool.tile([128, NJ], F32, tag="mmf")
        nc.vector.scalar_tensor_tensor(
            out=mm_f[:], in0=sg1[:], scalar=0.5, in1=vmul0[:],
            op0=ALU.add, op1=ALU.add,
        )
        ma_f = spool.tile([128, NJ], F32, tag="maf")
        nc.vector.tensor_add(ma_f[:], zb1[:, 8:16], zb0[:, 8:16])

        # ---- pass 3 ----
        addsum = spool.tile([128, 2], F32, tag="addsum")
        nc.vector.reduce_sum(
            out=addsum[:],
            in_=ma_f[:].rearrange("p (l t) -> p t l", t=2),
            axis=AX.X,
        )
        dpool = ctx.enter_context(tc.tile_pool(name="diag", bufs=1))
        diags = []
        for js in range(NJ):
            dt_ = dpool.tile([128, 128], BF, tag=f"diag{js}", name=f"diag{js}")
            nc.vector.tensor_scalar_mul(dt_[:], rhsi[:], mm_f[:, js:js + 1])
            diags.append(dt_)

        with tc.tile_pool(name="psO", bufs=4, space=bass.MemorySpace.PSUM) as psO:
            for nch in range(NJ):
                for jj in range(2):
                    cols = slice(512 * nch, 512 * (nch + 1))
                    stg = stpool.tile([128, 512], F32, tag="stg")
                    if nch < 4:
                        # PE cells
                        ps_o = psO.tile([128, 512], F32, tag="big")
                        for lv in range(4):
                            js = 2 * lv + jj
                            nc.tensor.matmul(
                                ps_o[:], diags[js][:], x_sb[:, js, cols],
                                start=(lv == 0), stop=(lv == 3),
                            )
                        nc.scalar.activation(
                            stg[:], ps_o[:], AF.Identity,
                            bias=addsum[:, jj:jj + 1], scale=1.0,
                        )
                    else:
                        # Scalar does term 0 (+bias); DVE chains terms 1-3
                        acc = apool.tile([128, 2, 512], F32, tag="acc")
                        nc.scalar.activation(
                            acc[:, 0, :], x_sb[:, jj, cols], AF.Identity,
                            bias=addsum[:, jj:jj + 1],
                            scale=mm_f[:, jj:jj + 1],
                        )
                        for lv in range(1, 4):
                            js = 2 * lv + jj
                            nc.vector.scalar_tensor_tensor(
                                out=(stg[:] if lv == 3 else acc[:, lv % 2, :]),
                                in0=x_sb[:, js, cols],
                                scalar=mm_f[:, js:js + 1],
                                in1=acc[:, (lv - 1) % 2, :],
                                op0=ALU.mult, op1=ALU.add,
                            )
                    nc.sync.dma_start(
                        out_d[128 * jj:128 * (jj + 1), cols], stg[:],
                    )

    nc.compile()
    return nc


def _pack_inputs(x0, x1, x2, x3, wm, bm,
                 add_W1, add_b1, add_g, add_be, add_W2, add_b2,
                 mul_W1, mul_b1, mul_g, mul_be, mul_W2, mul_b2):
    bf = ml_dtypes.bfloat16
    f8 = ml_dtypes.float8_e4m3fn
    f32 = np.float32

    wmc = np.asarray(wm, f32).reshape(NJ, 128).T.astype(bf).copy()
    rhsi = np.eye(128, dtype=bf)
    onesr = np.ones((1, 128), bf)

    wg1 = np.zeros((R, NJ, 128, 2048), f8)
    wg2 = np.zeros((R, 128, 4096), f8)
    sm = np.zeros((128, 128), f32)
    w1scale = [SC_W1R0, SC_W1R1]
    for r in range(R):
        for br, (W1, W2, b1, g, be, b2) in enumerate([
            (mul_W1[r], mul_W2[r], mul_b1[r], mul_g[r], mul_be[r], mul_b2[r]),
            (add_W1[r], add_W2[r], add_b1[r], add_g[r], add_be[r], add_b2[r]),
        ]):
            w1 = np.asarray(W1, f32).reshape(C, C) * w1scale[r]   # [lp, c]
            t1 = w1.reshape(NJ, 128, NJ, 128)             # [m, p', j, q]
            t1 = t1.transpose(2, 3, 0, 1).reshape(NJ, 128, 1024)
            wg1[r, :, :, 1024 * br:1024 * (br + 1)] = t1.astype(f8)

            w2 = np.asarray(W2, f32) * SC_W2              # [lv, cl, p]
            t2 = w2.reshape(4, 2, 128, 2, 128)            # [lv, clc, cl', kc, q]
            t2 = t2.transpose(4, 0, 1, 3, 2).reshape(128, 2048)
            wg2[r, :, 2048 * br:2048 * (br + 1)] = t2.astype(f8)

            # b1 enters LN on the scaled h: scale it to match SC_H * h + b1'
            hscale = w1scale[r] * (SC_V0 if r == 0 else SC_V1)
            b1a = np.asarray(b1, f32).reshape(C).reshape(NJ, 128).T * hscale
            sm[:, 0 + 16 * r + 8 * br: 0 + 16 * r + 8 * br + 8] = b1a
            for arr, base in ((g, 32), (be, 64)):
                a = np.asarray(arr, f32).reshape(C).reshape(NJ, 128).T
                sm[:, base + 16 * r + 8 * br: base + 16 * r + 8 * br + 8] = a
            b2a = np.asarray(b2, f32).reshape(4, 2, 128).transpose(2, 0, 1).reshape(128, 8)
            sm[:, 96 + 16 * r + 8 * br: 96 + 16 * r + 8 * br + 8] = b2a

    shared = dict(wmc=wmc, rhsi=rhsi, onesr=onesr, smalls=sm, wg1=wg1, wg2=wg2)

    in_maps = []
    xs = [np.asarray(a, f32) for a in (x0, x1, x2, x3)]
    for b in range(B):
        xc = np.concatenate(
            [a[b].reshape(CL, HW) for a in xs], axis=0
        ).astype(bf)
        in_maps.append({"x": xc, **shared})
    return in_maps


def kernel(**inputs):
    from concourse.bass_utils import run_bass_kernel_spmd

    if "nc" not in _CACHE:
        _CACHE["nc"] = _build_nc()
    nc = _CACHE["nc"]

    in_maps = _pack_inputs(**inputs)
    res = run_bass_kernel_spmd(nc, in_maps, list(range(NCORES)))
    _CACHE["last_results"] = res
    out = np.stack(
        [res.results[b]["out"].reshape(CL, H, W) for b in range(B)]
    ).astype(np.float32)
    return out


# revision 10
# speedup vs baseline: 1.0743x; 1.0743x over previous
# Trainium2 Bass kernel for the ContextBlock problem.
#
# Reference computation (per sample b):
#   xc    = concat(x0..x3)            [C=1024, HW=4096]
#   attn  = softmax(wm @ xc)          [HW]
#   ctx   = xc @ attn                 [C]
#   mul   = residual-gated MLP stack (sigmoid branch)   [C]
#   add   = residual-gated MLP stack (linear branch)    [C]
#   out   = sum_l (x_l * mul_l + add_l)                 [CL=256, HW]
#
# Distribution: data-parallel over batch, one sample per NeuronCore (B=8).
#
# Per-core dataflow (v3):
#   x arrives in 4 column-blocks of 1024; per block: PE computes logits,
#   scalar exps them (unnormalized: the softmax scale cancels through the
#   gates' LayerNorm), PE broadcasts e across partitions, DVE+Scalar
#   accumulate u[c] = sum_n x[c,n] e[n]. u is finalized per c-slab so the
#   first gate matvecs overlap the phase-A drain.
#   Gates: both branches (mul/add) fused per repeat, weight-stationary
#   matvecs on PE, LN stats via gpsimd partition_all_reduce, sigmoid as a
#   degree-7 odd polynomial on DVE (no activation-table swaps; |z| < 2).
#   Pass3: out chunks split PE (diag-matmul) / Scalar+DVE (scalar chains).

import numpy as np
import ml_dtypes
from contextlib import ExitStack

import concourse.bass as bass
import concourse.bacc as bacc
import concourse.mybir as mybir
import concourse.tile as tile

BF = mybir.dt.bfloat16
F32 = mybir.dt.float32
AF = mybir.ActivationFunctionType
ALU = mybir.AluOpType
AX = mybir.AxisListType

B, L, CL, H, W = 8, 4, 256, 64, 64
C = L * CL          # 1024
HW = H * W          # 4096
P = C // 4          # 256
R = 2
EPS = 1e-5
NJ = C // 128       # 8   c-slabs
NBLK = 4            # x column blocks of 1024
BLKW = HW // NBLK   # 1024
NCORES = 8

# sigmoid(z) - 0.5 ~= (((d z^2 + c) z^2 + b) z^2 + a) * z  on |z|<3.5
SGA, SGB, SGC, SGD = 2.48367550e-01, -1.85723651e-02, 1.15603304e-03, -3.27830204e-05

_CACHE = {}


def _build_nc():
    import concourse.bass_isa as bass_isa

    nc = bacc.Bacc()

    x_d = nc.dram_tensor("x", [C, HW], BF, kind="ExternalInput")
    wmc_d = nc.dram_tensor("wmc", [128, NJ], BF, kind="ExternalInput")
    rhsi_d = nc.dram_tensor("rhsi", [128, 128], BF, kind="ExternalInput")
    onesr_d = nc.dram_tensor("onesr", [1, 128], BF, kind="ExternalInput")
    sm_d = nc.dram_tensor("smalls", [128, 128], F32, kind="ExternalInput")
    wg1_d = nc.dram_tensor("wg1", [R, NJ, 128, 2048], BF, kind="ExternalInput")
    wg2_d = nc.dram_tensor("wg2", [R, 128, 4096], BF, kind="ExternalInput")
    out_d = nc.dram_tensor("out", [CL, HW], F32, kind="ExternalOutput")

    with tile.TileContext(nc) as tc, ExitStack() as ctx:
        resid = ctx.enter_context(tc.tile_pool(name="resid", bufs=1))
        spool = ctx.enter_context(tc.tile_pool(name="spool", bufs=1))
        stpool = ctx.enter_context(tc.tile_pool(name="stage", bufs=6))
        apool = ctx.enter_context(tc.tile_pool(name="accp", bufs=2))

        # ---- resident tiles + loads (all weights resident) ----
        wmc = resid.tile([128, NJ], BF, tag="wmc")
        rhsi = resid.tile([128, 128], BF, tag="rhsi")
        onesr = resid.tile([1, 128], BF, tag="onesr")
        sm = resid.tile([128, 128], F32, tag="sm")
        x_sb = resid.tile([128, NJ, HW], BF, tag="x")
        wg1 = resid.tile([128, R, NJ, 2048], BF, tag="wg1")
        wg2 = resid.tile([128, R, 4096], BF, tag="wg2")

        nc.sync.dma_start(wmc[:], wmc_d[:])
        nc.sync.dma_start(rhsi[:], rhsi_d[:])
        nc.sync.dma_start(onesr[:], onesr_d[:])
        nc.sync.dma_start(sm[:], sm_d[:])
        for blk in range(NBLK):
            cols = slice(BLKW * blk, BLKW * (blk + 1))
            for j in range(NJ):
                nc.sync.dma_start(x_sb[:, j, cols], x_d[128 * j:128 * (j + 1), cols])
        for r in range(R):
            for j in range(NJ):
                nc.sync.dma_start(wg1[:, r, j, :], wg1_d[r, j])
            nc.sync.dma_start(wg2[:, r, :], wg2_d[r])

        # single Exp table load up front (Identity/Copy live in every table)
        warm = spool.tile([1, 2], F32, tag="warm")
        nc.vector.memset(warm[:], 0.25)
        nc.scalar.activation(warm[0:1, 1:2], warm[0:1, 0:1], AF.Exp)

        # ---- phase A: logits -> exp -> u accumulation, per column block ---
        e_row = spool.tile([1, HW], BF, tag="e_row")
        e_bc = spool.tile([128, NBLK, BLKW], BF, tag="e_bc")
        scr_v = spool.tile([128, 2, BLKW], BF, tag="scr_v")
        prod = spool.tile([128, 2, 3, BLKW], BF, tag="prod")
        scr_s = spool.tile([128, 2, BLKW], BF, tag="scr_s")
        u_parts = spool.tile([128, NJ * NBLK], F32, tag="u_parts")
        v0 = spool.tile([128, NJ], F32, tag="v0")
        v0_bf = spool.tile([128, NJ], BF, tag="v0bf")

        with tc.tile_pool(name="psA", bufs=2, space=bass.MemorySpace.PSUM) as psA:
            for blk in range(NBLK):
                cols = slice(BLKW * blk, BLKW * (blk + 1))
                lg = psA.tile([1, BLKW], F32, tag="lg")
                for j in range(NJ):
                    for h in range(2):
                        nc.tensor.matmul(
                            lg[0:1, 512 * h:512 * (h + 1)],
                            wmc[:, j:j + 1],
                            x_sb[:, j, BLKW * blk + 512 * h:BLKW * blk + 512 * (h + 1)],
                            start=(j == 0), stop=(j == NJ - 1),
                        )
                nc.scalar.activation(e_row[0:1, cols], lg[:], AF.Exp)
                for h in range(2):
                    bc_ps = psA.tile([128, 512], F32, tag="bc")
                    nc.tensor.matmul(
                        bc_ps[:], onesr[:],
                        e_row[0:1, BLKW * blk + 512 * h:BLKW * blk + 512 * (h + 1)],
                    )
                    nc.scalar.copy(
                        e_bc[:, blk, 512 * h:512 * (h + 1)], bc_ps[:]
                    )
                # u accumulation: slabs 0-4 fused STT on DVE;
                # slabs 5-7: 2x tensor_tensor on DVE + accum on Scalar
                for j in range(5):
                    nc.vector.scalar_tensor_tensor(
                        out=scr_v[:, blk % 2, :],
                        in0=x_sb[:, j, cols],
                        scalar=1.0,
                        in1=e_bc[:, blk, :],
                        op0=ALU.bypass,
                        op1=ALU.mult,
                        accum_out=u_parts[:, j * NBLK + blk:j * NBLK + blk + 1],
                    )
                    if blk == NBLK - 1:
                        nc.vector.reduce_sum(
                            out=v0[:, j:j + 1],
                            in_=u_parts[:, j * NBLK:(j + 1) * NBLK],
                            axis=AX.X,
                        )
                        nc.vector.tensor_copy(v0_bf[:, j:j + 1], v0[:, j:j + 1])
                for j in range(5, NJ):
                    nc.vector.tensor_mul(
                        prod[:, blk % 2, j - 5, :], x_sb[:, j, cols], e_bc[:, blk, :]
                    )
                    nc.scalar.activation(
                        scr_s[:, blk % 2, :], prod[:, blk % 2, j - 5, :], AF.Identity,
                        accum_out=u_parts[:, j * NBLK + blk:j * NBLK + blk + 1],
                    )
                    if blk == NBLK - 1:
                        nc.vector.reduce_sum(
                            out=v0[:, j:j + 1],
                            in_=u_parts[:, j * NBLK:(j + 1) * NBLK],
                            axis=AX.X,
                        )
                        nc.vector.tensor_copy(v0_bf[:, j:j + 1], v0[:, j:j + 1])

        # prefetch the sqrt table while W1-r0 runs (Identity stays resident)
        nc.scalar.activation(warm[0:1, 1:2], warm[0:1, 0:1], AF.Sqrt)

        # ---- gates: both branches fused per repeat ----
        # t-col layout: t = br*8 + 2*lv + half
        ps = ctx.enter_context(
            tc.tile_pool(name="psG", bufs=2, space=bass.MemorySpace.PSUM)
        )

        def sigmoid_m05(z, tagp):
            # returns sigma(z) - 0.5 via odd poly (DVE only)
            t = spool.tile([128, 4, NJ], F32, tag=tagp)
            nc.vector.tensor_mul(t[:, 0, :], z, z)                      # z2
            nc.vector.tensor_scalar(
                out=t[:, 1, :], in0=t[:, 0, :],
                scalar1=SGD, scalar2=SGC, op0=ALU.mult, op1=ALU.add,
            )
            nc.vector.tensor_mul(t[:, 2, :], t[:, 1, :], t[:, 0, :])
            nc.vector.tensor_scalar_add(t[:, 2, :], t[:, 2, :], SGB)
            nc.vector.tensor_mul(t[:, 3, :], t[:, 2, :], t[:, 0, :])
            out = spool.tile([128, NJ], F32, tag=tagp + "o")
            nc.vector.scalar_tensor_tensor(
                out=out[:], in0=t[:, 3, :], scalar=SGA, in1=z,
                op0=ALU.add, op1=ALU.mult,
            )
            return out

        def gate_repeat(r, moving):
            b1c = sm[:, 0 + 16 * r:16 + 16 * r]
            gc = sm[:, 32 + 16 * r:48 + 16 * r]
            bec = sm[:, 64 + 16 * r:80 + 16 * r]
            b2c = sm[:, 96 + 16 * r:112 + 16 * r]

            ps_h = ps.tile([128, 16], F32, tag="ps_h")
            for j in range(NJ):
                for t in range(16):
                    nc.tensor.matmul(
                        ps_h[:, t:t + 1],
                        wg1[:, r, j, 128 * t:128 * (t + 1)],
                        moving(j, t // 8),
                        start=(j == 0 and t == 0),
                        stop=(j == NJ - 1 and t == 15),
                    )

            # LN over (partitions x 2 t-cols) per group g = br*4+lv
            stats = spool.tile([128, 32], F32, tag="stats", bufs=2)
            nc.vector.tensor_add(stats[:, 0:16], ps_h[:], b1c)
            nc.vector.tensor_mul(stats[:, 16:32], stats[:, 0:16], stats[:, 0:16])

            allred = spool.tile([128, 32], F32, tag="allred", bufs=2)
            nc.gpsimd.partition_all_reduce(
                allred[:], stats[:], channels=128, reduce_op=bass_isa.ReduceOp.add
            )

            gm = spool.tile([128, 16], F32, tag="gm", bufs=2)
            nc.vector.reduce_sum(
                out=gm[:],
                in_=allred[:].rearrange("p (g h) -> p g h", h=2),
                axis=AX.X,
            )
            nc.vector.tensor_scalar_mul(gm[:], gm[:], 1.0 / P)
            nbc = spool.tile([128, 16], F32, tag="nbc", bufs=2)
            nc.vector.tensor_mul(nbc[:, 0:8], gm[:, 0:8], gm[:, 0:8])
            nc.vector.tensor_sub(nbc[:, 8:16], gm[:, 8:16], nbc[:, 0:8])
            nc.vector.tensor_scalar_add(nbc[:, 8:16], nbc[:, 8:16], EPS)
            nc.scalar.activation(nbc[:, 0:8], nbc[:, 8:16], AF.Sqrt)
            nc.vector.reciprocal(nbc[:, 8:16], nbc[:, 0:8])
            mu_t = spool.tile([128, 32], F32, tag="mu_t", bufs=2)
            mtv = mu_t[:, 0:16].rearrange("p (g h) -> p h g", h=2)
            rtv = mu_t[:, 16:32].rearrange("p (g h) -> p h g", h=2)
            for hh in range(2):
                nc.vector.tensor_copy(mtv[:, hh, :], gm[:, 0:8])
                nc.vector.tensor_copy(rtv[:, hh, :], nbc[:, 8:16])

            hn = spool.tile([128, 16], F32, tag="hn", bufs=2)
            nc.vector.tensor_sub(hn[:], stats[:, 0:16], mu_t[:, 0:16])
            nc.vector.tensor_mul(hn[:], hn[:], mu_t[:, 16:32])
            nc.vector.tensor_mul(hn[:], hn[:], gc)
            nc.vector.tensor_add(hn[:], hn[:], bec)
            hn_bf = spool.tile([128, 16], BF, tag="hnbf", bufs=2)
            nc.vector.tensor_scalar_max(hn_bf[:], hn[:], 0.0)

            ps_z = ps.tile([128, 16], F32, tag="ps_z")
            nblks = 0
            for br in range(2):
                for lv in range(4):
                    for clc in range(2):
                        tcol = br * 8 + 2 * lv + clc
                        for kc in range(2):
                            off = (((br * 4 + lv) * 2 + clc) * 2 + kc) * 128
                            nc.tensor.matmul(
                                ps_z[:, tcol:tcol + 1],
                                wg2[:, r, off:off + 128],
                                hn_bf[:, br * 8 + 2 * lv + kc:br * 8 + 2 * lv + kc + 1],
                                start=(nblks == 0),
                                stop=(nblks == 31),
                            )
                            nblks += 1
            zb = spool.tile([128, 16], F32, tag=f"zb{r}")
            nc.vector.tensor_add(zb[:], ps_z[:], b2c)
            return zb

        zb0 = gate_repeat(0, lambda j, br: v0_bf[:, j:j + 1])

        sg0 = sigmoid_m05(zb0[:, 0:8], "sg0")          # sigma(z_mul0) - 0.5
        vmul0 = spool.tile([128, NJ], F32, tag="vmul0")
        nc.vector.tensor_scalar_add(vmul0[:], sg0[:], 0.5)
        v1_bf = spool.tile([128, 16], BF, tag="v1bf")
        nc.vector.tensor_copy(v1_bf[:, 0:8], vmul0[:])
        nc.vector.tensor_copy(v1_bf[:, 8:16], zb0[:, 8:16])

        zb1 = gate_repeat(1, lambda j, br: v1_bf[:, br * 8 + j:br * 8 + j + 1])

        sg1 = sigmoid_m05(zb1[:, 0:8], "sg1")
        mm_f = spool.tile([128, NJ], F32, tag="mmf")
        nc.vector.scalar_tensor_tensor(
            out=mm_f[:], in0=sg1[:], scalar=0.5, in1=vmul0[:],
            op0=ALU.add, op1=ALU.add,
        )
        ma_f = spool.tile([128, NJ], F32, tag="maf")
        nc.vector.tensor_add(ma_f[:], zb1[:, 8:16], zb0[:, 8:16])

        # ---- pass 3 ----
        addsum = spool.tile([128, 2], F32, tag="addsum")
        nc.vector.reduce_sum(
            out=addsum[:],
            in_=ma_f[:].rearrange("p (l t) -> p t l", t=2),
            axis=AX.X,
        )
        dpool = ctx.enter_context(tc.tile_pool(name="diag", bufs=1))
        diags = []
        for js in range(NJ):
            dt_ = dpool.tile([128, 128], BF, tag=f"diag{js}", name=f"diag{js}")
            nc.vector.tensor_scalar_mul(dt_[:], rhsi[:], mm_f[:, js:js + 1])
            diags.append(dt_)

        # 16 cells (nch, jj): 11 on PE, 5 on Scalar+DVE
        dve_cells = {(5, 1), (6, 0), (6, 1), (7, 0), (7, 1)}
        with tc.tile_pool(name="psO", bufs=4, space=bass.MemorySpace.PSUM) as psO:
            for nch in range(NJ):
                for jj in range(2):
                    cols = slice(512 * nch, 512 * (nch + 1))
                    stg = stpool.tile([128, 512], F32, tag="stg")
                    if (nch, jj) not in dve_cells:
                        ps_o = psO.tile([128, 512], F32, tag="big")
                        for lv in range(4):
                            js = 2 * lv + jj
                            nc.tensor.matmul(
                                ps_o[:], diags[js][:], x_sb[:, js, cols],
                                start=(lv == 0), stop=(lv == 3),
                            )
                        nc.scalar.activation(
                            stg[:], ps_o[:], AF.Identity,
                            bias=addsum[:, jj:jj + 1], scale=1.0,
                        )
                    else:
                        # Scalar does term 0 (+bias); DVE chains terms 1-3
                        acc = apool.tile([128, 2, 512], F32, tag="acc")
                        nc.scalar.activation(
                            acc[:, 0, :], x_sb[:, jj, cols], AF.Identity,
                            bias=addsum[:, jj:jj + 1],
                            scale=mm_f[:, jj:jj + 1],
                        )
                        for lv in range(1, 4):
                            js = 2 * lv + jj
                            nc.vector.scalar_tensor_tensor(
                                out=(stg[:] if lv == 3 else acc[:, lv % 2, :]),
                                in0=x_sb[:, js, cols],
                                scalar=mm_f[:, js:js + 1],
                                in1=acc[:, (lv - 1) % 2, :],
                                op0=ALU.mult, op1=ALU.add,
                            )
                    nc.sync.dma_start(
                        out_d[128 * jj:128 * (jj + 1), cols], stg[:],
                    )

    nc.compile()
    return nc


def _pack_inputs(x0, x1, x2, x3, wm, bm,
                 add_W1, add_b1, add_g, add_be, add_W2, add_b2,
                 mul_W1, mul_b1, mul_g, mul_be, mul_W2, mul_b2):
    bf = ml_dtypes.bfloat16
    f32 = np.float32

    wmc = np.asarray(wm, f32).reshape(NJ, 128).T.astype(bf).copy()
    rhsi = np.eye(128, dtype=bf)
    onesr = np.ones((1, 128), bf)

    wg1 = np.zeros((R, NJ, 128, 2048), bf)
    wg2 = np.zeros((R, 128, 4096), bf)
    sm = np.zeros((128, 128), f32)
    for r in range(R):
        for br, (W1, W2, b1, g, be, b2) in enumerate([
            (mul_W1[r], mul_W2[r], mul_b1[r], mul_g[r], mul_be[r], mul_b2[r]),
            (add_W1[r], add_W2[r], add_b1[r], add_g[r], add_be[r], add_b2[r]),
        ]):
            w1 = np.asarray(W1, f32).reshape(C, C)       # [lp, c]
            t1 = w1.reshape(NJ, 128, NJ, 128)             # [m, p', j, q]
            t1 = t1.transpose(2, 3, 0, 1).reshape(NJ, 128, 1024)
            wg1[r, :, :, 1024 * br:1024 * (br + 1)] = t1.astype(bf)

            w2 = np.asarray(W2, f32)                      # [lv, cl, p]
            t2 = w2.reshape(4, 2, 128, 2, 128)            # [lv, clc, cl', kc, q]
            t2 = t2.transpose(4, 0, 1, 3, 2).reshape(128, 2048)
            wg2[r, :, 2048 * br:2048 * (br + 1)] = t2.astype(bf)

            for arr, base in ((b1, 0), (g, 32), (be, 64)):
                a = np.asarray(arr, f32).reshape(C).reshape(NJ, 128).T
                sm[:, base + 16 * r + 8 * br: base + 16 * r + 8 * br + 8] = a
            b2a = np.asarray(b2, f32).reshape(4, 2, 128).transpose(2, 0, 1).reshape(128, 8)
            sm[:, 96 + 16 * r + 8 * br: 96 + 16 * r + 8 * br + 8] = b2a

    shared = dict(wmc=wmc, rhsi=rhsi, onesr=onesr, smalls=sm, wg1=wg1, wg2=wg2)

    in_maps = []
    xs = [np.asarray(a, f32) for a in (x0, x1, x2, x3)]
    for b in range(B):
        xc = np.concatenate(
            [a[b].reshape(CL, HW) for a in xs], axis=0
        ).astype(bf)
        in_maps.append({"x": xc, **shared})
    return in_maps


def kernel(**inputs):
    from concourse.bass_utils import run_bass_kernel_spmd

    if "nc" not in _CACHE:
        _CACHE["nc"] = _build_nc()
    nc = _CACHE["nc"]

    in_maps = _pack_inputs(**inputs)
    res = run_bass_kernel_spmd(nc, in_maps, list(range(NCORES)))
    _CACHE["last_results"] = res
    out = np.stack(
        [res.results[b]["out"].reshape(CL, H, W) for b in range(B)]
    ).astype(np.float32)
    return out


# revision 11
# speedup vs baseline: 1.1054x; 1.0289x over previous
# Trainium2 Bass kernel for the ContextBlock problem.
#
# Reference computation (per sample b):
#   xc    = concat(x0..x3)            [C=1024, HW=4096]
#   attn  = softmax(wm @ xc)          [HW]
#   ctx   = xc @ attn                 [C]
#   mul   = residual-gated MLP stack (sigmoid branch)   [C]
#   add   = residual-gated MLP stack (linear branch)    [C]
#   out   = sum_l (x_l * mul_l + add_l)                 [CL=256, HW]
#
# Distribution: data-parallel over batch, one sample per NeuronCore (B=8).
#
# Per-core dataflow (v4):
#   Few, large DMAs (descriptor-gen on the sync engine costs ~700ns each).
#   x arrives in 4 column-blocks of 1024; per block: PE computes logits per
#   512-half, scalar exps each half (unnormalized: softmax scale cancels
#   through the gates' LayerNorm), PE broadcasts e, and u[c] = sum x[c,n]e[n]
#   accumulates split across DVE (fused STT), Pool (products) and Scalar
#   (activation-accumulate). v0 finalizes per c-slab on Scalar so the first
#   gate matvecs overlap the phase-A drain.
#   Gates: both branches fused per repeat, weight-stationary matvecs on PE,
#   LN stats via gpsimd partition_all_reduce, sigmoid as a degree-7 odd
#   polynomial on DVE (no activation-table swaps; |z| < 2 in practice).
#   Pass3: out chunks split PE (diag-matmul) / Scalar+DVE (scalar chains).

import numpy as np
import ml_dtypes
from contextlib import ExitStack

import concourse.bass as bass
import concourse.bacc as bacc
import concourse.mybir as mybir
import concourse.tile as tile

BF = mybir.dt.bfloat16
F32 = mybir.dt.float32
AF = mybir.ActivationFunctionType
ALU = mybir.AluOpType
AX = mybir.AxisListType

B, L, CL, H, W = 8, 4, 256, 64, 64
C = L * CL          # 1024
HW = H * W          # 4096
P = C // 4          # 256
R = 2
EPS = 1e-5
NJ = C // 128       # 8   c-slabs
NBLK = 4            # x column blocks of 1024
BLKW = HW // NBLK   # 1024
NCORES = 8

# sigmoid(z) - 0.5 ~= (((d z^2 + c) z^2 + b) z^2 + a) * z  on |z|<3.5
SGA, SGB, SGC, SGD = 2.48367550e-01, -1.85723651e-02, 1.15603304e-03, -3.27830204e-05

_CACHE = {}


def _build_nc():
    import concourse.bass_isa as bass_isa

    nc = bacc.Bacc()

    x_d = nc.dram_tensor("x", [C, HW], BF, kind="ExternalInput")
    wmc_d = nc.dram_tensor("wmc", [128, NJ], BF, kind="ExternalInput")
    rhsi_d = nc.dram_tensor("rhsi", [128, 128], BF, kind="ExternalInput")
    onesr_d = nc.dram_tensor("onesr", [1, 128], BF, kind="ExternalInput")
    sm_d = nc.dram_tensor("smalls", [128, 128], F32, kind="ExternalInput")
    wg1_d = nc.dram_tensor("wg1", [R, NJ, 128, 2048], BF, kind="ExternalInput")
    wg2_d = nc.dram_tensor("wg2", [R, 128, 4096], BF, kind="ExternalInput")
    out_d = nc.dram_tensor("out", [CL, HW], F32, kind="ExternalOutput")

    with tile.TileContext(nc) as tc, ExitStack() as ctx:
        resid = ctx.enter_context(tc.tile_pool(name="resid", bufs=1))
        spool = ctx.enter_context(tc.tile_pool(name="spool", bufs=1))
        stpool = ctx.enter_context(tc.tile_pool(name="stage", bufs=6))
        apool = ctx.enter_context(tc.tile_pool(name="accp", bufs=2))

        # ---- resident tiles + merged loads ----
        wmc = resid.tile([128, NJ], BF, tag="wmc")
        rhsi = resid.tile([128, 128], BF, tag="rhsi")
        onesr = resid.tile([1, 128], BF, tag="onesr")
        sm = resid.tile([128, 128], F32, tag="sm")
        x_sb = resid.tile([128, NJ, HW], BF, tag="x")
        wg1 = resid.tile([128, R, NJ, 2048], BF, tag="wg1")
        wg2 = resid.tile([128, R, 4096], BF, tag="wg2")

        nc.sync.dma_start(wmc[:], wmc_d[:])
        nc.sync.dma_start(rhsi[:], rhsi_d[:])
        nc.sync.dma_start(onesr[:], onesr_d[:])
        nc.sync.dma_start(sm[:], sm_d[:])
        xv = x_d[:].rearrange("(j p) n -> p j n", p=128)
        for blk in range(NBLK):
            cols = slice(BLKW * blk, BLKW * (blk + 1))
            nc.sync.dma_start(x_sb[:, :, cols], xv[:, :, cols])
        for r in range(R):
            nc.sync.dma_start(
                wg1[:, r, :, :], wg1_d[r].rearrange("j q w -> q j w")
            )
            nc.sync.dma_start(wg2[:, r, :], wg2_d[r])

        # single Exp table load up front (Identity/Copy live in every table)
        warm = spool.tile([1, 2], F32, tag="warm")
        nc.vector.memset(warm[:], 0.25)
        nc.scalar.activation(warm[0:1, 1:2], warm[0:1, 0:1], AF.Exp)

        # ---- phase A ----
        e_row = spool.tile([1, HW], BF, tag="e_row")
        e_bc = spool.tile([128, NBLK, BLKW], BF, tag="e_bc")
        scr_v = spool.tile([128, 2, BLKW], BF, tag="scr_v")
        prod = spool.tile([128, 2, 3, BLKW], BF, tag="prod")
        scr_s = spool.tile([128, 2, BLKW], BF, tag="scr_s")
        u_parts = spool.tile([128, NJ * NBLK], F32, tag="u_parts")
        v0c = spool.tile([128, NJ], F32, tag="v0c")
        v0_bf = spool.tile([128, NJ], BF, tag="v0bf")
        scr40 = spool.tile([128, 4], F32, tag="scr40")

        with tc.tile_pool(name="psA", bufs=2, space=bass.MemorySpace.PSUM) as psA:
            for blk in range(NBLK):
                cols = slice(BLKW * blk, BLKW * (blk + 1))
                lg = psA.tile([1, BLKW], F32, tag="lg")
                for h in range(2):
                    for j in range(NJ):
                        nc.tensor.matmul(
                            lg[0:1, 512 * h:512 * (h + 1)],
                            wmc[:, j:j + 1],
                            x_sb[:, j, BLKW * blk + 512 * h:BLKW * blk + 512 * (h + 1)],
                            start=(j == 0), stop=(j == NJ - 1),
                        )
                    nc.scalar.activation(
                        e_row[0:1, BLKW * blk + 512 * h:BLKW * blk + 512 * (h + 1)],
                        lg[0:1, 512 * h:512 * (h + 1)], AF.Exp,
                    )
                for h in range(2):
                    bc_ps = psA.tile([128, 512], F32, tag="bc")
                    nc.tensor.matmul(
                        bc_ps[:], onesr[:],
                        e_row[0:1, BLKW * blk + 512 * h:BLKW * blk + 512 * (h + 1)],
                    )
                    nc.scalar.copy(
                        e_bc[:, blk, 512 * h:512 * (h + 1)], bc_ps[:]
                    )
                # u accumulation: slabs 0-4 fused STT on DVE; slab 5 product
                # on DVE + accum on Scalar; slabs 6-7 product on Pool + Scalar
                for j in range(5):
                    nc.vector.scalar_tensor_tensor(
                        out=scr_v[:, blk % 2, :],
                        in0=x_sb[:, j, cols],
                        scalar=1.0,
                        in1=e_bc[:, blk, :],
                        op0=ALU.bypass,
                        op1=ALU.mult,
                        accum_out=u_parts[:, j * NBLK + blk:j * NBLK + blk + 1],
                    )
                for j in range(5, NJ):
                    eng = nc.vector if j == 5 else nc.gpsimd
                    eng.tensor_mul(
                        prod[:, blk % 2, j - 5, :], x_sb[:, j, cols], e_bc[:, blk, :]
                    )
                    nc.scalar.activation(
                        scr_s[:, blk % 2, :], prod[:, blk % 2, j - 5, :], AF.Identity,
                        accum_out=u_parts[:, j * NBLK + blk:j * NBLK + blk + 1],
                    )
                if blk == NBLK - 1:
                    # finalize v0 per c-slab on Scalar (keeps DVE free and
                    # lets gate W1 groups start during the drain)
                    for j in range(NJ):
                        nc.scalar.activation(
                            scr40[:], u_parts[:, j * NBLK:(j + 1) * NBLK],
                            AF.Identity, accum_out=v0c[:, j:j + 1],
                        )
                        nc.scalar.activation(
                            v0_bf[:, j:j + 1], v0c[:, j:j + 1], AF.Identity
                        )

        # prefetch the sqrt table while W1-r0 runs (Identity stays resident)
        nc.scalar.activation(warm[0:1, 1:2], warm[0:1, 0:1], AF.Sqrt)

        # ---- gates ----
        ps = ctx.enter_context(
            tc.tile_pool(name="psG", bufs=2, space=bass.MemorySpace.PSUM)
        )

        def sigmoid_m05(z, tagp):
            t = spool.tile([128, 4, NJ], F32, tag=tagp)
            nc.vector.tensor_mul(t[:, 0, :], z, z)
            nc.vector.tensor_scalar(
                out=t[:, 1, :], in0=t[:, 0, :],
                scalar1=SGD, scalar2=SGC, op0=ALU.mult, op1=ALU.add,
            )
            nc.vector.tensor_mul(t[:, 2, :], t[:, 1, :], t[:, 0, :])
            nc.vector.tensor_scalar_add(t[:, 2, :], t[:, 2, :], SGB)
            nc.vector.tensor_mul(t[:, 3, :], t[:, 2, :], t[:, 0, :])
            out = spool.tile([128, NJ], F32, tag=tagp + "o")
            nc.vector.scalar_tensor_tensor(
                out=out[:], in0=t[:, 3, :], scalar=SGA, in1=z,
                op0=ALU.add, op1=ALU.mult,
            )
            return out

        def gate_repeat(r, moving):
            b1c = sm[:, 0 + 16 * r:16 + 16 * r]
            gc = sm[:, 32 + 16 * r:48 + 16 * r]
            bec = sm[:, 64 + 16 * r:80 + 16 * r]
            b2c = sm[:, 96 + 16 * r:112 + 16 * r]

            ps_h = ps.tile([128, 16], F32, tag="ps_h")
            for j in range(NJ):
                for t in range(16):
                    nc.tensor.matmul(
                        ps_h[:, t:t + 1],
                        wg1[:, r, j, 128 * t:128 * (t + 1)],
                        moving(j, t // 8),
                        start=(j == 0 and t == 0),
                        stop=(j == NJ - 1 and t == 15),
                    )

            stats = spool.tile([128, 32], F32, tag="stats", bufs=2)
            nc.vector.tensor_add(stats[:, 0:16], ps_h[:], b1c)
            nc.vector.tensor_mul(stats[:, 16:32], stats[:, 0:16], stats[:, 0:16])

            allred = spool.tile([128, 32], F32, tag="allred", bufs=2)
            nc.gpsimd.partition_all_reduce(
                allred[:], stats[:], channels=128, reduce_op=bass_isa.ReduceOp.add
            )

            gm = spool.tile([128, 16], F32, tag="gm", bufs=2)
            nc.vector.reduce_sum(
                out=gm[:],
                in_=allred[:].rearrange("p (g h) -> p g h", h=2),
                axis=AX.X,
            )
            nc.vector.tensor_scalar_mul(gm[:], gm[:], 1.0 / P)
            nbc = spool.tile([128, 16], F32, tag="nbc", bufs=2)
            nc.vector.tensor_mul(nbc[:, 0:8], gm[:, 0:8], gm[:, 0:8])
            nc.vector.tensor_sub(nbc[:, 8:16], gm[:, 8:16], nbc[:, 0:8])
            nc.vector.tensor_scalar_add(nbc[:, 8:16], nbc[:, 8:16], EPS)
            nc.scalar.activation(nbc[:, 0:8], nbc[:, 8:16], AF.Sqrt)
            nc.vector.reciprocal(nbc[:, 8:16], nbc[:, 0:8])
            mu_t = spool.tile([128, 32], F32, tag="mu_t", bufs=2)
            mtv = mu_t[:, 0:16].rearrange("p (g h) -> p h g", h=2)
            rtv = mu_t[:, 16:32].rearrange("p (g h) -> p h g", h=2)
            for hh in range(2):
                nc.vector.tensor_copy(mtv[:, hh, :], gm[:, 0:8])
                nc.vector.tensor_copy(rtv[:, hh, :], nbc[:, 8:16])

            hn = spool.tile([128, 16], F32, tag="hn", bufs=2)
            nc.vector.tensor_sub(hn[:], stats[:, 0:16], mu_t[:, 0:16])
            nc.vector.tensor_mul(hn[:], hn[:], mu_t[:, 16:32])
            nc.vector.tensor_mul(hn[:], hn[:], gc)
            nc.vector.tensor_add(hn[:], hn[:], bec)
            hn_bf = spool.tile([128, 16], BF, tag="hnbf", bufs=2)
            nc.vector.tensor_scalar_max(hn_bf[:], hn[:], 0.0)

            ps_z = ps.tile([128, 16], F32, tag="ps_z")
            nblks = 0
            for br in range(2):
                for lv in range(4):
                    for clc in range(2):
                        tcol = br * 8 + 2 * lv + clc
                        for kc in range(2):
                            off = (((br * 4 + lv) * 2 + clc) * 2 + kc) * 128
                            nc.tensor.matmul(
                                ps_z[:, tcol:tcol + 1],
                                wg2[:, r, off:off + 128],
                                hn_bf[:, br * 8 + 2 * lv + kc:br * 8 + 2 * lv + kc + 1],
                                start=(nblks == 0),
                                stop=(nblks == 31),
                            )
                            nblks += 1
            zb = spool.tile([128, 16], F32, tag=f"zb{r}")
            nc.vector.tensor_add(zb[:], ps_z[:], b2c)
            return zb

        zb0 = gate_repeat(0, lambda j, br: v0_bf[:, j:j + 1])

        sg0 = sigmoid_m05(zb0[:, 0:8], "sg0")
        vmul0 = spool.tile([128, NJ], F32, tag="vmul0")
        nc.vector.tensor_scalar_add(vmul0[:], sg0[:], 0.5)
        v1_bf = spool.tile([128, 16], BF, tag="v1bf")
        nc.vector.tensor_copy(v1_bf[:, 0:8], vmul0[:])
        nc.vector.tensor_copy(v1_bf[:, 8:16], zb0[:, 8:16])

        zb1 = gate_repeat(1, lambda j, br: v1_bf[:, br * 8 + j:br * 8 + j + 1])

        sg1 = sigmoid_m05(zb1[:, 0:8], "sg1")
        mm_f = spool.tile([128, NJ], F32, tag="mmf")
        nc.vector.scalar_tensor_tensor(
            out=mm_f[:], in0=sg1[:], scalar=0.5, in1=vmul0[:],
            op0=ALU.add, op1=ALU.add,
        )
        ma_f = spool.tile([128, NJ], F32, tag="maf")
        nc.vector.tensor_add(ma_f[:], zb1[:, 8:16], zb0[:, 8:16])

        # ---- pass 3 ----
        addsum = spool.tile([128, 2], F32, tag="addsum")
        nc.vector.reduce_sum(
            out=addsum[:],
            in_=ma_f[:].rearrange("p (l t) -> p t l", t=2),
            axis=AX.X,
        )
        dpool = ctx.enter_context(tc.tile_pool(name="diag", bufs=1))
        diags = []
        for js in range(NJ):
            dt_ = dpool.tile([128, 128], BF, tag=f"diag{js}", name=f"diag{js}")
            nc.vector.tensor_scalar_mul(dt_[:], rhsi[:], mm_f[:, js:js + 1])
            diags.append(dt_)

        dve_cells = {(5, 1), (6, 0), (6, 1), (7, 0), (7, 1)}
        with tc.tile_pool(name="psO", bufs=4, space=bass.MemorySpace.PSUM) as psO:
            for nch in range(NJ):
                for jj in range(2):
                    cols = slice(512 * nch, 512 * (nch + 1))
                    stg = stpool.tile([128, 512], F32, tag="stg")
                    if (nch, jj) not in dve_cells:
                        ps_o = psO.tile([128, 512], F32, tag="big")
                        for lv in range(4):
                            js = 2 * lv + jj
                            nc.tensor.matmul(
                                ps_o[:], diags[js][:], x_sb[:, js, cols],
                                start=(lv == 0), stop=(lv == 3),
                            )
                        nc.scalar.activation(
                            stg[:], ps_o[:], AF.Identity,
                            bias=addsum[:, jj:jj + 1], scale=1.0,
                        )
                    else:
                        acc = apool.tile([128, 2, 512], F32, tag="acc")
                        nc.scalar.activation(
                            acc[:, 0, :], x_sb[:, jj, cols], AF.Identity,
                            bias=addsum[:, jj:jj + 1],
                            scale=mm_f[:, jj:jj + 1],
                        )
                        for lv in range(1, 4):
                            js = 2 * lv + jj
                            nc.vector.scalar_tensor_tensor(
                                out=(stg[:] if lv == 3 else acc[:, lv % 2, :]),
                                in0=x_sb[:, js, cols],
                                scalar=mm_f[:, js:js + 1],
                                in1=acc[:, (lv - 1) % 2, :],
                                op0=ALU.mult, op1=ALU.add,
                            )
                    nc.sync.dma_start(
                        out_d[128 * jj:128 * (jj + 1), cols], stg[:],
                    )

    nc.compile()
    return nc


def _pack_inputs(x0, x1, x2, x3, wm, bm,
                 add_W1, add_b1, add_g, add_be, add_W2, add_b2,
                 mul_W1, mul_b1, mul_g, mul_be, mul_W2, mul_b2):
    bf = ml_dtypes.bfloat16
    f32 = np.float32

    wmc = np.asarray(wm, f32).reshape(NJ, 128).T.astype(bf).copy()
    rhsi = np.eye(128, dtype=bf)
    onesr = np.ones((1, 128), bf)

    wg1 = np.zeros((R, NJ, 128, 2048), bf)
    wg2 = np.zeros((R, 128, 4096), bf)
    sm = np.zeros((128, 128), f32)
    for r in range(R):
        for br, (W1, W2, b1, g, be, b2) in enumerate([
            (mul_W1[r], mul_W2[r], mul_b1[r], mul_g[r], mul_be[r], mul_b2[r]),
            (add_W1[r], add_W2[r], add_b1[r], add_g[r], add_be[r], add_b2[r]),
        ]):
            w1 = np.asarray(W1, f32).reshape(C, C)       # [lp, c]
            t1 = w1.reshape(NJ, 128, NJ, 128)             # [m, p', j, q]
            t1 = t1.transpose(2, 3, 0, 1).reshape(NJ, 128, 1024)
            wg1[r, :, :, 1024 * br:1024 * (br + 1)] = t1.astype(bf)

            w2 = np.asarray(W2, f32)                      # [lv, cl, p]
            t2 = w2.reshape(4, 2, 128, 2, 128)            # [lv, clc, cl', kc, q]
            t2 = t2.transpose(4, 0, 1, 3, 2).reshape(128, 2048)
            wg2[r, :, 2048 * br:2048 * (br + 1)] = t2.astype(bf)

            for arr, base in ((b1, 0), (g, 32), (be, 64)):
                a = np.asarray(arr, f32).reshape(C).reshape(NJ, 128).T
                sm[:, base + 16 * r + 8 * br: base + 16 * r + 8 * br + 8] = a
            b2a = np.asarray(b2, f32).reshape(4, 2, 128).transpose(2, 0, 1).reshape(128, 8)
            sm[:, 96 + 16 * r + 8 * br: 96 + 16 * r + 8 * br + 8] = b2a

    shared = dict(wmc=wmc, rhsi=rhsi, onesr=onesr, smalls=sm, wg1=wg1, wg2=wg2)

    in_maps = []
    xs = [np.asarray(a, f32) for a in (x0, x1, x2, x3)]
    for b in range(B):
        xc = np.concatenate(
            [a[b].reshape(CL, HW) for a in xs], axis=0
        ).astype(bf)
        in_maps.append({"x": xc, **shared})
    return in_maps


def kernel(**inputs):
    from concourse.bass_utils import run_bass_kernel_spmd

    if "nc" not in _CACHE:
        _CACHE["nc"] = _build_nc()
    nc = _CACHE["nc"]

    in_maps = _pack_inputs(**inputs)
    res = run_bass_kernel_spmd(nc, in_maps, list(range(NCORES)))
    _CACHE["last_results"] = res
    out = np.stack(
        [res.results[b]["out"].reshape(CL, H, W) for b in range(B)]
    ).astype(np.float32)
    return out
